# revision 19
# baseline (speedup 1.0000x reference)
"""MoE expert FFN (swiglu) kernel for 8 trn2 NeuronCores.

Expert parallelism: 8 experts, one per core. Each core computes, for its
expert e:
    h   = x_e @ w1_e            # [2048, 2048] @ [2048, 2816]
    act = silu(h[:, :1408]) * h[:, 1408:]
    out = act @ w2_e            # [2048, 1408] @ [1408, 2048]

Tokens arrive pre-sorted by expert with equal counts (2048/expert), so
sharding is a static slice and the gather is a concat. No collectives.

Device-side layout (bf16 compute, fp32 PSUM accumulation, bf16 out that
the host upcasts):
  mm1: out[f, t] tiles; lhsT = w1 128x128 tiles (stationary),
       rhs = xT[h, t] (moving, N=512) -> inter is [f, t], the layout mm2
       needs, so no on-device transpose anywhere (x is transposed on host).
  swiglu pairs: psum tile f-block j (a) with f-block j+11 (b);
       act_j = silu(a) * b  via ACT(Silu) + DVE mul -> bf16 SBUF.
  mm2: out[t, h] tiles, n-outer/k-inner so each PSUM block's copy+store
       overlaps the remaining matmuls; lhsT = act[f, t] 128-col slices,
       rhs = w2[f, h] (moving, N=512). PSUM -> SBUF bf16 -> DMA out.

The kernel is PE-bound: 2112 matmuls x 512 cols at 1 col/cycle bf16
(~456us at 2.37GHz); everything else is about keeping the PE fed:
  - fp8 DoubleRow was evaluated and rejected: plain fp8 rel-err ~6.6e-2
    fails the 2e-2 gate, and the 3-term hi/lo-compensated variant (which
    passes at ~2e-3) needs 1.5x the matmul work vs DoubleRow's ~1.5-1.8x
    speedup -> no win on TRN2 silicon.
  - w1 is prepacked on host into swiglu-paired column slabs
    [p, pair j, k-group, {a|b}, 128] so one contiguous DMA delivers BOTH
    halves of a pair for all k (whole-row tiles stalled pair 0 ~27us
    waiting for b-half columns). First slab is k-granular (65KB) so the
    first LDWEIGHTS fires ~10us in.
  - DMA issue blocks the issuing ENGINE (~0.6us/DMA + backpressure), so
    engines with compute roles stay clear of bulk loads: sync carries
    w1-even/w2/outs, scalar only 4 tiny x0 head tiles (ACT must be free
    by ~16us or silu(pair0) blocks PSUM-bank rotation at pair 4), gpsimd
    (SWDGE) carries x0 tail, w1-odd, then x chunks 1-3 behind w1 in queue
    FIFO order. x and w2 are host-packed so chunks load as 4-5 coarse
    strided DMAs instead of 11-16.
  - out is stored bf16: halves store traffic and the post-matmul drain.
Residual overhead is ~10us NEFF preamble + first-transfer, ~6us early
delivery-bound trickle (queues ramp from ~50GB/s), ~12us semaphore
teardown postamble. Measured 483us vs the 511us whole-row baseline;
rel-err 4.4e-3.
"""

import os
import sys

sys.path.insert(0, "/opt/trn_rl_repo")

import numpy as np
import ml_dtypes

E = 8             # experts == cores
T_TOTAL = 16384
H = 2048
F = 1408
F2 = 2 * F        # 2816
TPC = T_TOTAL // E  # 2048 tokens per core
CHUNK = 512
NCH = TPC // CHUNK          # 4 chunks
KH = H // 128               # 16 contraction tiles for mm1
NF = F // 128               # 11 f-blocks per half (a / b)
NT = CHUNK // 128           # 4 m-tiles per chunk in mm2
NHO = H // 512              # 4 output column blocks
KG = 4                      # w1 k-groups per pair slab (4 k-tiles each)
GW = (KH // KG) * 256       # 1024 cols per k-group slab

_CACHE = {}

# Optional knobs read by test.py (not used by the grading harness).
TRACE = os.environ.get("BASS_TRACE_KERNEL", "0") == "1"
LAST = {}


def _build():
    from concourse import bacc, tile, mybir

    bf16 = mybir.dt.bfloat16
    f32 = mybir.dt.float32
    SILU = mybir.ActivationFunctionType.Silu

    # Bacc (not plain Bass): its lowering pipeline splits multi-sem waits
    # into EventSemaphore pairs — TRN2 allows at most 1 wait per instruction.
    nc = bacc.Bacc()
    xT_d = nc.declare_dram_parameter("xT", [128, KH, TPC], bf16, isOutput=False)
    # w1 prepacked on host: [128, 11 pairs * 16 k * (a|b) * 128] — see kernel().
    w1_d = nc.declare_dram_parameter("w1", [128, NF * KH * 256], bf16, isOutput=False)
    w2_d = nc.declare_dram_parameter("w2", [128, NF, H], bf16, isOutput=False)
    out_d = nc.declare_dram_parameter("out", [TPC, H], bf16, isOutput=True)

    with tile.TileContext(nc) as tc:
        with (
            tc.tile_pool(name="w1p", bufs=1) as w1p,
            tc.tile_pool(name="w2p", bufs=1) as w2p,
            tc.tile_pool(name="x0p", bufs=1) as x0p,
            tc.tile_pool(name="xp", bufs=2) as xp,
            tc.tile_pool(name="actp", bufs=1) as actp,
            tc.tile_pool(name="tmpp", bufs=2) as tmpp,
            tc.tile_pool(name="outp", bufs=4) as outp,
            tc.tile_pool(name="psp", bufs=8, space="PSUM") as psp,
        ):
            # DMA issue blocks the issuing ENGINE (~0.6us/DMA + queue
            # backpressure for the whole transfer backlog), so engines with
            # compute roles must stay clear of bulk loads:
            #   sync (HWDGE):   w1-even slabs, then w2, then out stores.
            #   scalar (HWDGE): 4 tiny x0 head tiles ONLY — ACT must be free
            #                   by ~16us or silu(pair0) blocks the PSUM-bank
            #                   rotation at pair 4 (v2 lost 19us to this).
            #   gpsimd (SWDGE): x0 tail, w1-odd, x chunks 1-3. FIFO order
            #                   keeps w1 ahead of the deferrable x chunks.
            #   vector: no DMA — its muls also gate PSUM reuse.
            # PE warm-up: ~36 dependency-free matmuls (~7.7us) on a zeroed
            # dummy tile, never read. The real first matmul waits ~9.5us for
            # DMA-queue spin-up + first transfers; without this the HAM
            # power-state ramp only starts then and early matmuls run ~15%
            # slow until ~27us. Warm-up starts the ramp at ~1us instead.
            warm = w1p.tile([128, CHUNK], bf16, tag="warm", name="warm")
            nc.vector.memset(warm[:], 0.0)
            pw = psp.tile([128, CHUNK], f32, tag="ps", name="warm_ps")
            for i in range(36):
                nc.tensor.matmul(
                    pw[:], warm[:, 0:128], warm[:], start=(i == 0), stop=(i == 35)
                )

            x0_t = []
            for k in range(KH):
                t = x0p.tile([128, 1, CHUNK], bf16, tag=f"x_{k}", name=f"x0_{k}")
                x0_t.append(t)

            # Slab (0,0) is four k-granular tiles so the first LDWEIGHTS
            # waits on a 65KB transfer, not 262KB.
            w1_t = [[None] * (KH // KG) for _ in range(NF)]
            w100 = [
                w1p.tile([128, 256], bf16, tag=f"w1_00{h}", name=f"w1_00{h}")
                for h in range(KG)
            ]
            for j in range(NF):
                for g in range(KH // KG):
                    if j == 0 and g == 0:
                        continue
                    w1_t[j][g] = w1p.tile(
                        [128, GW], bf16, tag=f"w1_{j}_{g}", name=f"w1_{j}_{g}"
                    )

            def w1_slice(j, k, half):
                # stationary [128, 128] for (pair j, k-tile k, a/b half)
                if j == 0 and k < KG:
                    return w100[k][:, half * 128 : half * 128 + 128]
                o = (k % KG) * 256 + half * 128
                return w1_t[j][k // KG][:, o : o + 128]

            # Early phase is delivery-bound (queues ramp from ~50GB/s): spread
            # x0 over FOUR queues (vector joins as a pure DMA issuer — its
            # first mul isn't needed until ~25us) and order pair-0's w1 slabs
            # by PE consumption: (0,2) feeds a-half k8-11, (0,1)/(0,3) the
            # b-half.
            def w1_dma(eng, j, g):
                c0 = j * (KH * 256) + g * GW
                eng.dma_start(out=w1_t[j][g][:], in_=w1_d[:, c0 : c0 + GW])

            for h in range(KG):
                nc.sync.dma_start(
                    out=w100[h][:], in_=w1_d[:, h * 256 : (h + 1) * 256]
                )
            for k in range(4):
                nc.scalar.dma_start(out=x0_t[k][:], in_=xT_d[:, k : k + 1, 0:CHUNK])
            for k in range(4, KH):
                nc.gpsimd.dma_start(out=x0_t[k][:], in_=xT_d[:, k : k + 1, 0:CHUNK])
            for j in range(NF):
                for g in range(KH // KG):
                    if j == 0 and g == 0:
                        continue
                    eng = nc.sync if (j * (KH // KG) + g) % 2 == 0 else nc.gpsimd
                    w1_dma(eng, j, g)

            # Resident w2: 11 tiles [128, 2048]; not needed until mm2 of
            # chunk 0 (~120us in) — behind w1-even on the sync queue.
            w2_t = w2p.tile([128, NF, H], bf16, tag="w2", name="w2")
            nc.sync.dma_start(out=w2_t[:], in_=w2_d[:])

            for c in range(NCH):
                # Stream this chunk of tokens (columns of xT); chunk 0 was
                # preloaded above. gpsimd keeps the HWDGE queues clear.
                if c == 0:
                    x_t = [x0_t[k][:, 0, :] for k in range(KH)]
                else:
                    xg = []
                    for g in range(KH // KG):
                        t = xp.tile(
                            [128, KG, CHUNK], bf16, tag=f"xg_{g}", name=f"x_{c}_{g}"
                        )
                        xg.append(t)
                        nc.gpsimd.dma_start(
                            out=t[:],
                            in_=xT_d[:, g * KG : (g + 1) * KG,
                                     c * CHUNK : (c + 1) * CHUNK],
                        )
                    x_t = [xg[k // KG][:, k % KG, :] for k in range(KH)]

                # mm1 + swiglu, one (a, b) f-block pair at a time.
                act_t = []
                for j in range(NF):
                    ps_a = psp.tile([128, CHUNK], f32, tag="ps")
                    ps_b = psp.tile([128, CHUNK], f32, tag="ps")
                    for k in range(KH):
                        nc.tensor.matmul(
                            ps_a[:],
                            w1_slice(j, k, 0),
                            x_t[k],
                            start=(k == 0),
                            stop=(k == KH - 1),
                        )
                    for k in range(KH):
                        nc.tensor.matmul(
                            ps_b[:],
                            w1_slice(j, k, 1),
                            x_t[k],
                            start=(k == 0),
                            stop=(k == KH - 1),
                        )
                    tmp = tmpp.tile([128, CHUNK], f32, tag="tmp")
                    nc.scalar.activation(tmp[:], ps_a[:], SILU)
                    a = actp.tile([128, CHUNK], bf16, tag=f"act_{j}")
                    act_t.append(a)
                    nc.vector.tensor_mul(a[:], tmp[:], ps_b[:])

                # mm2: out[t, h] for this chunk. n-outer / k-inner: each
                # n-block's PSUM completes early, so its copy + store overlap
                # the remaining matmuls (shrinks the end-of-kernel drain).
                for m in range(NT):
                    r0 = c * CHUNK + m * 128
                    for n in range(NHO):
                        po = psp.tile([128, 512], f32, tag="ps", name=f"po_{c}_{m}_{n}")
                        for k in range(NF):
                            nc.tensor.matmul(
                                po[:],
                                act_t[k][:, m * 128 : (m + 1) * 128],
                                w2_t[:, k, n * 512 : (n + 1) * 512],
                                start=(k == 0),
                                stop=(k == NF - 1),
                            )
                        osb = outp.tile([128, 512], bf16, tag="osb")
                        nc.scalar.copy(osb[:], po[:])
                        nc.sync.dma_start(
                            out=out_d[r0 : r0 + 128, n * 512 : (n + 1) * 512],
                            in_=osb[:],
                        )
    if not nc.is_finalized():
        nc.finalize()  # Bacc.finalize runs the lowering pipeline (sem split, alloc_regs)
    return nc


def _get_nc():
    if "nc" not in _CACHE:
        _CACHE["nc"] = _build()
    return _CACHE["nc"]


def _pack_w1(w1e):
    """[H, 2F] f32 -> [128, 11*16*2*128] bf16, swiglu-paired column slabs.

    Layout: col index = ((j * KH + k) * 2 + half) * 128 + c, holding
    w1e[k*128 + p, (j + half*NF) * 128 + c] at partition row p.
    """
    a = w1e.reshape(KH, 128, 2 * NF, 128)            # k, p, fb, c
    t = a.transpose(1, 2, 0, 3)                      # p, fb, k, c
    pairs = np.stack([t[:, :NF], t[:, NF:]], axis=3)  # p, j, k, half, c
    return np.ascontiguousarray(
        pairs.reshape(128, NF * KH * 256).astype(ml_dtypes.bfloat16)
    )


def kernel(permuted_hidden_states, num_tokens_per_expert, w1, w2):
    from concourse.bass_utils import run_bass_kernel_spmd

    x = np.asarray(permuted_hidden_states, dtype=np.float32)
    w1 = np.asarray(w1, dtype=np.float32)
    w2 = np.asarray(w2, dtype=np.float32)
    ntpe = np.asarray(num_tokens_per_expert)
    assert x.shape == (T_TOTAL, H) and w1.shape == (E, H, F2) and w2.shape == (E, F, H)
    # Reference semantics rely on the static equal split.
    assert np.all(ntpe == TPC), f"expected equal {TPC}-token splits, got {ntpe}"

    bf = ml_dtypes.bfloat16
    in_maps = []
    for e in range(E):
        xe = x[e * TPC : (e + 1) * TPC]
        xT = xe.T.reshape(KH, 128, TPC).transpose(1, 0, 2)
        w2p = w2[e].reshape(NF, 128, H).transpose(1, 0, 2)
        in_maps.append(
            {
                "xT": np.ascontiguousarray(xT).astype(bf),
                "w1": _pack_w1(w1[e]),
                "w2": np.ascontiguousarray(w2p).astype(bf),
            }
        )

    nc = _get_nc()
    res = run_bass_kernel_spmd(nc, in_maps, list(range(E)), trace=TRACE)
    LAST["exec_time_ns"] = res.exec_time_ns
    LAST["mean_exec_time_ns"] = res.mean_exec_time_ns
    LAST["profile_json"] = res.profile_json
    out = np.concatenate(
        [np.asarray(res.results[i]["out"], dtype=np.float32) for i in range(E)], axis=0
    )
    return np.ascontiguousarray(out)


# revision 21
# speedup vs baseline: 1.0187x; 1.0187x over previous
"""MoE expert FFN (swiglu) kernel for 8 trn2 NeuronCores.

Expert parallelism: 8 experts, one per core. Each core computes, for its
expert e:
    h   = x_e @ w1_e            # [2048, 2048] @ [2048, 2816]
    act = silu(h[:, :1408]) * h[:, 1408:]
    out = act @ w2_e            # [2048, 1408] @ [1408, 2048]

Tokens arrive pre-sorted by expert with equal counts (2048/expert), so
sharding is a static slice and the gather is a concat. No collectives.

Device-side layout (bf16 compute, fp32 PSUM accumulation, bf16 out that
the host upcasts):
  mm1: out[f, t] tiles; lhsT = w1 128x128 tiles (stationary),
       rhs = xT[h, t] (moving, N=512) -> inter is [f, t], the layout mm2
       needs, so no on-device transpose anywhere (x is transposed on host).
  swiglu pairs: psum tile f-block j (a) with f-block j+11 (b);
       act_j = silu(a) * b  via ACT(Silu) + DVE mul -> bf16 SBUF.
  mm2: out[t, h] tiles, n-outer/k-inner so each PSUM block's copy+store
       overlaps the remaining matmuls; lhsT = act[f, t] 128-col slices,
       rhs = w2[f, h] (moving, N=512). PSUM -> SBUF bf16 -> DMA out.

The kernel is PE-bound: 2112 matmuls x 512 cols at 1 col/cycle bf16
(~456us at 2.37GHz); everything else is about keeping the PE fed:
  - fp8 DoubleRow was evaluated and rejected: plain fp8 rel-err ~6.6e-2
    fails the 2e-2 gate, and the 3-term hi/lo-compensated variant (which
    passes at ~2e-3) needs 1.5x the matmul work vs DoubleRow's ~1.5-1.8x
    speedup -> no win on TRN2 silicon.
  - w1 is prepacked on host into swiglu-paired column slabs
    [p, pair j, k-group, {a|b}, 128] so one contiguous DMA delivers BOTH
    halves of a pair for all k (whole-row tiles stalled pair 0 ~27us
    waiting for b-half columns). First slab is k-granular (65KB) so the
    first LDWEIGHTS fires ~10us in.
  - DMA issue blocks the issuing ENGINE (~0.6us/DMA + backpressure), so
    engines with compute roles stay clear of bulk loads: sync carries
    w1-even/w2/outs, scalar only 4 tiny x0 head tiles (ACT must be free
    by ~16us or silu(pair0) blocks PSUM-bank rotation at pair 4), gpsimd
    (SWDGE) carries x0 tail, w1-odd, then x chunks 1-3 behind w1 in queue
    FIFO order. x and w2 are host-packed so chunks load as 4-5 coarse
    strided DMAs instead of 11-16.
  - out is stored bf16: halves store traffic and the post-matmul drain.
Residual overhead is ~10us NEFF preamble + first-transfer, ~6us early
delivery-bound trickle (queues ramp from ~50GB/s), ~12us semaphore
teardown postamble. Measured 483us vs the 511us whole-row baseline;
rel-err 4.4e-3.
"""

import os
import sys

sys.path.insert(0, "/opt/trn_rl_repo")

import numpy as np
import ml_dtypes

E = 8             # experts == cores
T_TOTAL = 16384
H = 2048
F = 1408
F2 = 2 * F        # 2816
TPC = T_TOTAL // E  # 2048 tokens per core
CHUNK = 512
NCH = TPC // CHUNK          # 4 chunks
KH = H // 128               # 16 contraction tiles for mm1
NF = F // 128               # 11 f-blocks per half (a / b)
NT = CHUNK // 128           # 4 m-tiles per chunk in mm2
NHO = H // 512              # 4 output column blocks
KG = 4                      # w1 k-groups per pair slab (4 k-tiles each)
GW = (KH // KG) * 256       # 1024 cols per k-group slab

_CACHE = {}

# Optional knobs read by test.py (not used by the grading harness).
TRACE = os.environ.get("BASS_TRACE_KERNEL", "0") == "1"
LAST = {}


def _build():
    from concourse import bacc, tile, mybir

    bf16 = mybir.dt.bfloat16
    f32 = mybir.dt.float32
    SILU = mybir.ActivationFunctionType.Silu

    # Bacc (not plain Bass): its lowering pipeline splits multi-sem waits
    # into EventSemaphore pairs — TRN2 allows at most 1 wait per instruction.
    nc = bacc.Bacc()
    xT_d = nc.declare_dram_parameter("xT", [128, KH, TPC], bf16, isOutput=False)
    # w1 prepacked on host: [128, 11 pairs * 16 k * (a|b) * 128] — see kernel().
    w1_d = nc.declare_dram_parameter("w1", [128, NF * KH * 256], bf16, isOutput=False)
    w2_d = nc.declare_dram_parameter("w2", [128, NF, H], bf16, isOutput=False)
    out_d = nc.declare_dram_parameter("out", [TPC, H], bf16, isOutput=True)

    with tile.TileContext(nc) as tc:
        with (
            tc.tile_pool(name="w1p", bufs=1) as w1p,
            tc.tile_pool(name="w2p", bufs=1) as w2p,
            tc.tile_pool(name="x0p", bufs=1) as x0p,
            tc.tile_pool(name="xp", bufs=2) as xp,
            tc.tile_pool(name="actp", bufs=1) as actp,
            tc.tile_pool(name="tmpp", bufs=2) as tmpp,
            tc.tile_pool(name="outp", bufs=4) as outp,
            tc.tile_pool(name="psp", bufs=8, space="PSUM") as psp,
        ):
            # DMA issue blocks the issuing ENGINE (~0.6us/DMA + queue
            # backpressure for the whole transfer backlog), so engines with
            # compute roles must stay clear of bulk loads:
            #   sync (HWDGE):   w1-even slabs, then w2, then out stores.
            #   scalar (HWDGE): 4 tiny x0 head tiles ONLY — ACT must be free
            #                   by ~16us or silu(pair0) blocks the PSUM-bank
            #                   rotation at pair 4 (v2 lost 19us to this).
            #   gpsimd (SWDGE): x0 tail, w1-odd, x chunks 1-3. FIFO order
            #                   keeps w1 ahead of the deferrable x chunks.
            #   vector: no DMA — its muls also gate PSUM reuse.
            # PE warm-up: 12 dependency-free matmuls on a zeroed dummy tile,
            # never read. The real first matmul waits ~9.5us for DMA-queue
            # spin-up; the HAM power-state ramp only starts when the PE runs,
            # so early real matmuls pay ~15-20% reduced clock until ~27us.
            # Warm-up starts the ramp at ~1us. Sized in COLD cycles
            # (~550-650ns/MM at low power state -> ~7us): a 36-MM version
            # overshot data arrival by ~10us and regressed.
            warm = w1p.tile([128, CHUNK], bf16, tag="warm", name="warm")
            nc.vector.memset(warm[:], 0.0)
            pw = psp.tile([128, CHUNK], f32, tag="ps", name="warm_ps")
            for i in range(12):
                nc.tensor.matmul(
                    pw[:], warm[:, 0:128], warm[:], start=(i == 0), stop=(i == 11)
                )

            x0_t = []
            for k in range(KH):
                t = x0p.tile([128, 1, CHUNK], bf16, tag=f"x_{k}", name=f"x0_{k}")
                x0_t.append(t)

            # Slab (0,0) is four k-granular tiles so the first LDWEIGHTS
            # waits on a 65KB transfer, not 262KB.
            w1_t = [[None] * (KH // KG) for _ in range(NF)]
            w100 = [
                w1p.tile([128, 256], bf16, tag=f"w1_00{h}", name=f"w1_00{h}")
                for h in range(KG)
            ]
            for j in range(NF):
                for g in range(KH // KG):
                    if j == 0 and g == 0:
                        continue
                    w1_t[j][g] = w1p.tile(
                        [128, GW], bf16, tag=f"w1_{j}_{g}", name=f"w1_{j}_{g}"
                    )

            def w1_slice(j, k, half):
                # stationary [128, 128] for (pair j, k-tile k, a/b half)
                if j == 0 and k < KG:
                    return w100[k][:, half * 128 : half * 128 + 128]
                o = (k % KG) * 256 + half * 128
                return w1_t[j][k // KG][:, o : o + 128]

            # Early phase is delivery-bound (queues ramp from ~50GB/s): spread
            # x0 over FOUR queues (vector joins as a pure DMA issuer — its
            # first mul isn't needed until ~25us) and order pair-0's w1 slabs
            # by PE consumption: (0,2) feeds a-half k8-11, (0,1)/(0,3) the
            # b-half.
            def w1_dma(eng, j, g):
                c0 = j * (KH * 256) + g * GW
                eng.dma_start(out=w1_t[j][g][:], in_=w1_d[:, c0 : c0 + GW])

            for h in range(KG):
                nc.sync.dma_start(
                    out=w100[h][:], in_=w1_d[:, h * 256 : (h + 1) * 256]
                )
            for k in range(4):
                nc.scalar.dma_start(out=x0_t[k][:], in_=xT_d[:, k : k + 1, 0:CHUNK])
            for k in range(4, KH):
                nc.gpsimd.dma_start(out=x0_t[k][:], in_=xT_d[:, k : k + 1, 0:CHUNK])
            for j in range(NF):
                for g in range(KH // KG):
                    if j == 0 and g == 0:
                        continue
                    eng = nc.sync if (j * (KH // KG) + g) % 2 == 0 else nc.gpsimd
                    w1_dma(eng, j, g)

            # Resident w2: 11 tiles [128, 2048]; not needed until mm2 of
            # chunk 0 (~120us in) — behind w1-even on the sync queue.
            w2_t = w2p.tile([128, NF, H], bf16, tag="w2", name="w2")
            nc.sync.dma_start(out=w2_t[:], in_=w2_d[:])

            for c in range(NCH):
                # Stream this chunk of tokens (columns of xT); chunk 0 was
                # preloaded above. gpsimd keeps the HWDGE queues clear.
                if c == 0:
                    x_t = [x0_t[k][:, 0, :] for k in range(KH)]
                else:
                    xg = []
                    for g in range(KH // KG):
                        t = xp.tile(
                            [128, KG, CHUNK], bf16, tag=f"xg_{g}", name=f"x_{c}_{g}"
                        )
                        xg.append(t)
                        nc.gpsimd.dma_start(
                            out=t[:],
                            in_=xT_d[:, g * KG : (g + 1) * KG,
                                     c * CHUNK : (c + 1) * CHUNK],
                        )
                    x_t = [xg[k // KG][:, k % KG, :] for k in range(KH)]

                # mm1 + swiglu, one (a, b) f-block pair at a time.
                act_t = []
                for j in range(NF):
                    ps_a = psp.tile([128, CHUNK], f32, tag="ps")
                    ps_b = psp.tile([128, CHUNK], f32, tag="ps")
                    for k in range(KH):
                        nc.tensor.matmul(
                            ps_a[:],
                            w1_slice(j, k, 0),
                            x_t[k],
                            start=(k == 0),
                            stop=(k == KH - 1),
                        )
                    for k in range(KH):
                        nc.tensor.matmul(
                            ps_b[:],
                            w1_slice(j, k, 1),
                            x_t[k],
                            start=(k == 0),
                            stop=(k == KH - 1),
                        )
                    tmp = tmpp.tile([128, CHUNK], f32, tag="tmp")
                    nc.scalar.activation(tmp[:], ps_a[:], SILU)
                    a = actp.tile([128, CHUNK], bf16, tag=f"act_{j}")
                    act_t.append(a)
                    nc.vector.tensor_mul(a[:], tmp[:], ps_b[:])

                # mm2: out[t, h] for this chunk. n-outer / k-inner: each
                # n-block's PSUM completes early, so its copy + store overlap
                # the remaining matmuls (shrinks the end-of-kernel drain).
                for m in range(NT):
                    r0 = c * CHUNK + m * 128
                    for n in range(NHO):
                        po = psp.tile([128, 512], f32, tag="ps", name=f"po_{c}_{m}_{n}")
                        for k in range(NF):
                            nc.tensor.matmul(
                                po[:],
                                act_t[k][:, m * 128 : (m + 1) * 128],
                                w2_t[:, k, n * 512 : (n + 1) * 512],
                                start=(k == 0),
                                stop=(k == NF - 1),
                            )
                        osb = outp.tile([128, 512], bf16, tag="osb")
                        nc.scalar.copy(osb[:], po[:])
                        nc.sync.dma_start(
                            out=out_d[r0 : r0 + 128, n * 512 : (n + 1) * 512],
                            in_=osb[:],
                        )
    if not nc.is_finalized():
        nc.finalize()  # Bacc.finalize runs the lowering pipeline (sem split, alloc_regs)
    return nc


def _get_nc():
    if "nc" not in _CACHE:
        _CACHE["nc"] = _build()
    return _CACHE["nc"]


def _pack_w1(w1e):
    """[H, 2F] f32 -> [128, 11*16*2*128] bf16, swiglu-paired column slabs.

    Layout: col index = ((j * KH + k) * 2 + half) * 128 + c, holding
    w1e[k*128 + p, (j + half*NF) * 128 + c] at partition row p.
    """
    a = w1e.reshape(KH, 128, 2 * NF, 128)            # k, p, fb, c
    t = a.transpose(1, 2, 0, 3)                      # p, fb, k, c
    pairs = np.stack([t[:, :NF], t[:, NF:]], axis=3)  # p, j, k, half, c
    return np.ascontiguousarray(
        pairs.reshape(128, NF * KH * 256).astype(ml_dtypes.bfloat16)
    )


def kernel(permuted_hidden_states, num_tokens_per_expert, w1, w2):
    from concourse.bass_utils import run_bass_kernel_spmd

    x = np.asarray(permuted_hidden_states, dtype=np.float32)
    w1 = np.asarray(w1, dtype=np.float32)
    w2 = np.asarray(w2, dtype=np.float32)
    ntpe = np.asarray(num_tokens_per_expert)
    assert x.shape == (T_TOTAL, H) and w1.shape == (E, H, F2) and w2.shape == (E, F, H)
    # Reference semantics rely on the static equal split.
    assert np.all(ntpe == TPC), f"expected equal {TPC}-token splits, got {ntpe}"

    bf = ml_dtypes.bfloat16
    in_maps = []
    for e in range(E):
        xe = x[e * TPC : (e + 1) * TPC]
        xT = xe.T.reshape(KH, 128, TPC).transpose(1, 0, 2)
        w2p = w2[e].reshape(NF, 128, H).transpose(1, 0, 2)
        in_maps.append(
            {
                "xT": np.ascontiguousarray(xT).astype(bf),
                "w1": _pack_w1(w1[e]),
                "w2": np.ascontiguousarray(w2p).astype(bf),
            }
        )

    nc = _get_nc()
    res = run_bass_kernel_spmd(nc, in_maps, list(range(E)), trace=TRACE)
    LAST["exec_time_ns"] = res.exec_time_ns
    LAST["mean_exec_time_ns"] = res.mean_exec_time_ns
    LAST["profile_json"] = res.profile_json
    out = np.concatenate(
        [np.asarray(res.results[i]["out"], dtype=np.float32) for i in range(E)], axis=0
    )
    return np.ascontiguousarray(out)
